# revision 6
# baseline (speedup 1.0000x reference)
"""CharRNN (2-layer tanh RNN, B=256 T=200 H=2048 V=512) on 8 trn2 NeuronCores.

Strategy: tensor-parallel over the hidden dim (each core owns a 256-row slice
of every weight's output dim), state kept transposed [H, B] in fp16, one fused
AllGather of [h0_t ; h1_{t-1}] per timestep (fp16 bytes packed in f32-typed
buffers — f32-typed collectives are ~2x faster per byte than f16-typed).

The layer-0 input transform is collapsed via the vocab table:
  x_t @ W_ih0^T = E0[tokens_t]  with  E0 = emb @ W_ih0^T  (V=512 rows only),
realized as a one-hot matmul so it lands in the same PSUM accumulation as the
recurrent term. Biases ride the tanh activation's per-partition bias port.
Logits are computed in-loop (V sliced across cores) during AllGather flight.
"""

import numpy as np
import ml_dtypes

import concourse.bass as bass
import concourse.mybir as mybir
import concourse.tile as tile
from concourse.bass import ts

F16 = mybir.dt.float16
F32 = mybir.dt.float32
NPF16 = np.float16

H = 2048
B = 256
T_FULL = 200
V = 512
NC = 8
S = H // NC          # 256 hidden rows per core
KC = H // 128        # 16 contraction chunks
SC = S // 128        # 2 psum chunks per slice
VC = V // 128        # 4 vocab chunks
VS = V // NC         # 64 vocab rows per core (in-loop logits)


# ---------------------------------------------------------------- runner ----

def _legalize_waits(nc, max_waits=1):
    """Old walrus rejects >1 sem-wait per instruction; split extras onto
    preceding Drain instructions on the same engine."""
    n_split = 0
    for bb in nc.main_func.blocks:
        il = bb.instructions
        i = 0
        while i < len(il):
            inst = il[i]
            si = inst.sync_info
            if si is not None and len(si.on_wait) > max_waits:
                waits = list(si.on_wait)
                chunks = [waits[j:j + max_waits] for j in range(0, len(waits), max_waits)]
                for k, ch in enumerate(chunks[:-1]):
                    es = mybir.InstDrain(name=f"lw_{n_split}_{k}", ins=[], outs=[])
                    es.engine = inst.engine
                    es.sync_info = mybir.SyncInfo(on_update=[], on_wait=ch)
                    il.insert(i, es)
                    i += 1
                inst.sync_info = mybir.SyncInfo(
                    on_update=list(si.on_update), on_wait=chunks[-1])
                n_split += 1
            i += 1
    return n_split


def _make_runner(nc, n_cores):
    import jax
    from jax.sharding import Mesh, PartitionSpec
    from jax.experimental.shard_map import shard_map
    from concourse import bass2jax

    bass2jax.install_neuronx_cc_hook()
    partition_name = nc.partition_id_tensor.name if nc.partition_id_tensor else None

    in_names, out_names, out_avals, zero_outs = [], [], [], []
    for alloc in nc.m.functions[0].allocations:
        if not isinstance(alloc, mybir.MemoryLocationSet):
            continue
        name = alloc.memorylocations[0].name
        if alloc.kind == "ExternalInput":
            if name != partition_name:
                in_names.append(name)
        elif alloc.kind == "ExternalOutput":
            out_names.append(name)
            shape = tuple(alloc.tensor_shape)
            dtype = mybir.dt.np(alloc.dtype)
            out_avals.append(jax.core.ShapedArray(shape, dtype))
            zero_outs.append(np.zeros(shape, dtype))
    n_params = len(in_names)
    n_outs = len(out_avals)
    all_in_names = list(in_names) + list(out_names)
    if partition_name is not None:
        all_in_names.append(partition_name)
    donate = tuple(range(n_params, n_params + n_outs))

    def _body(*args):
        operands = list(args)
        if partition_name is not None:
            operands.append(bass2jax.partition_id_tensor())
        outs = bass2jax._bass_exec_p.bind(
            *operands,
            out_avals=tuple(out_avals),
            in_names=tuple(all_in_names),
            out_names=tuple(out_names),
            lowering_input_output_aliases=(),
            sim_require_finite=True,
            sim_require_nnan=True,
            nc=nc,
        )
        return tuple(outs)

    devices = jax.devices()[:n_cores]
    assert len(devices) == n_cores
    mesh = Mesh(np.asarray(devices), ("core",))
    sharded = jax.jit(
        shard_map(_body, mesh=mesh,
                  in_specs=(PartitionSpec("core"),) * (n_params + n_outs),
                  out_specs=(PartitionSpec("core"),) * n_outs,
                  check_rep=False),
        donate_argnums=donate, keep_unused=True)

    def fn(in_maps):
        import jax as _jax
        concat_in = [
            np.concatenate([np.asarray(in_maps[c][nm]) for c in range(n_cores)], axis=0)
            for nm in in_names]
        concat_zeros = [
            np.zeros((n_cores * z.shape[0], *z.shape[1:]), z.dtype) for z in zero_outs]
        out_arrs = sharded(*concat_in, *concat_zeros)
        _jax.block_until_ready(out_arrs)
        return [
            {nm: np.asarray(out_arrs[i]).reshape(n_cores, *out_avals[i].shape)[c]
             for i, nm in enumerate(out_names)}
            for c in range(n_cores)]

    return fn


# ---------------------------------------------------------------- kernel ----

def build_nc(T=T_FULL, legalize=True):
    nc = bass.Bass()

    whh0t = nc.dram_tensor("whh0t", [H, S], F16, kind="ExternalInput")
    wih1t = nc.dram_tensor("wih1t", [H, S], F16, kind="ExternalInput")
    whh1t = nc.dram_tensor("whh1t", [H, S], F16, kind="ExternalInput")
    wih0t = nc.dram_tensor("wih0t", [H, S], F16, kind="ExternalInput")
    embt = nc.dram_tensor("embt", [H, V], F16, kind="ExternalInput")
    fcwt = nc.dram_tensor("fcwt", [H, VS], F16, kind="ExternalInput")
    b0c = nc.dram_tensor("b0c", [128, SC], F32, kind="ExternalInput")
    b1c = nc.dram_tensor("b1c", [128, SC], F32, kind="ExternalInput")
    fbc = nc.dram_tensor("fbc", [VS, 1], F32, kind="ExternalInput")
    hin0 = nc.dram_tensor("hin0", [H, B], F16, kind="ExternalInput")
    hin1 = nc.dram_tensor("hin1", [H, B], F16, kind="ExternalInput")
    hin1s = nc.dram_tensor("hin1s", [128, SC, B], F16, kind="ExternalInput")
    tokf = nc.dram_tensor("tokf", [T, 128, B], F16, kind="ExternalInput")
    iotat = nc.dram_tensor("iotat", [128, VC], F16, kind="ExternalInput")

    logits_out = nc.dram_tensor("logits_out", [T, VS, B], F32, kind="ExternalOutput")
    hfin_out = nc.dram_tensor("hfin_out", [2, S, B], F32, kind="ExternalOutput")

    rg = [list(range(NC))]

    with tile.TileContext(nc) as tc:
        with (
            tc.tile_pool(name="wp", bufs=1) as wp,
            tc.tile_pool(name="gp", bufs=2) as gp,
            tc.tile_pool(name="sp", bufs=3) as sp,
            tc.tile_pool(name="pp", bufs=2, space="PSUM") as pp,
            tc.tile_pool(name="dp", bufs=3, space="DRAM") as dp,
        ):
            # ---- persistent weights ----
            w_whh0 = wp.tile([128, KC, S], F16, name="w_whh0")
            w_wih1 = wp.tile([128, KC, S], F16, name="w_wih1")
            w_whh1 = wp.tile([128, KC, S], F16, name="w_whh1")
            w_wih0 = wp.tile([128, KC, S], F16, name="w_wih0")
            w_embt = wp.tile([128, KC, V], F16, name="w_embt")
            w_fcw = wp.tile([128, KC, VS], F16, name="w_fcw")
            for dst, src in ((w_whh0, whh0t), (w_wih1, wih1t), (w_whh1, whh1t),
                             (w_wih0, wih0t), (w_embt, embt), (w_fcw, fcwt)):
                nc.sync.dma_start(dst[:], src.rearrange("(ko ki) s -> ki ko s", ki=128))
            b0_sb = wp.tile([128, SC], F32, name="b0_sb")
            b1_sb = wp.tile([128, SC], F32, name="b1_sb")
            fb_sb = wp.tile([VS, 1], F32, name="fb_sb")
            iota_sb = wp.tile([128, VC], F16, name="iota_sb")
            nc.sync.dma_start(b0_sb[:], b0c[:])
            nc.sync.dma_start(b1_sb[:], b1c[:])
            nc.sync.dma_start(fb_sb[:], fbc[:])
            nc.sync.dma_start(iota_sb[:], iotat[:])

            # ---- E0 = emb @ W_ih0^T (this core's slice), f16 [128, VC, S] ----
            e0_sb = wp.tile([128, VC, S], F16, name="e0_sb")
            for v in range(VC):
                pe0 = pp.tile([128, S], F32, tag="ps_e0")
                for k in range(KC):
                    nc.tensor.matmul(pe0[:], w_embt[:, k, ts(v, 128)], w_wih0[:, k, :],
                                     start=(k == 0), stop=(k == KC - 1))
                nc.scalar.activation(e0_sb[:, v, :], pe0[:],
                                     mybir.ActivationFunctionType.Copy)

            # ---- one-hot builder (DVE), one tick ahead ----
            def build_oh(t):
                tokr = sp.tile([128, B], F16, tag="tokr", name="tokr")
                nc.sync.dma_start(tokr[:], tokf[t])
                oh = sp.tile([128, VC, B], F16, tag="oh", name="oh")
                for v in range(VC):
                    nc.vector.tensor_tensor(
                        oh[:, v, :],
                        iota_sb[:, v:v + 1].to_broadcast((128, B)),
                        tokr[:],
                        mybir.AluOpType.is_equal)
                return oh

            # ---- initial h1 slice (for AG_0's h1 half) ----
            h1s = sp.tile([128, SC, B], F16, tag="h1s", name="h1s_init")
            nc.sync.dma_start(h1s[:], hin1s[:])
            h0s = None
            hf_done = [False, False]

            oh_cur = build_oh(0)

            agout_prev = None
            for tau in range(T + 2):
                # -- gathered state loads (from AG_{tau-1} / init) --
                if tau <= T:
                    h0f = gp.tile([128, KC, B], F16, tag="h0f", name="h0f")
                    if tau == 0:
                        nc.sync.dma_start(
                            h0f[:], hin0.rearrange("(ko ki) b -> ki ko b", ki=128))
                    else:
                        src = agout_prev.bitcast(F16).rearrange(
                            "(c l ci p) n -> l ci p c n", c=NC, l=2, ci=SC, p=128)
                        for ci in range(SC):
                            nc.sync.dma_start(
                                h0f[:].rearrange("p (c ci) b -> ci p c b", c=NC)[ci],
                                src[0, ci])
                if 1 <= tau <= T + 1:
                    h1f = gp.tile([128, KC, B], F16, tag="h1f", name="h1f")
                    if tau == 1:
                        nc.sync.dma_start(
                            h1f[:], hin1.rearrange("(ko ki) b -> ki ko b", ki=128))
                    else:
                        src = agout_prev.bitcast(F16).rearrange(
                            "(c l ci p) n -> l ci p c n", c=NC, l=2, ci=SC, p=128)
                        for ci in range(SC):
                            nc.sync.dma_start(
                                h1f[:].rearrange("p (c ci) b -> ci p c b", c=NC)[ci],
                                src[1, ci])

                # -- L0: h0_tau --
                if tau <= T - 1:
                    h0s = sp.tile([128, SC, B], F16, tag="h0s", name="h0s")
                    for m in range(SC):
                        p0 = pp.tile([128, B], F32, tag="ps_l0", name="ps_l0")
                        for v in range(VC):
                            nc.tensor.matmul(p0[:], e0_sb[:, v, ts(m, 128)],
                                             oh_cur[:, v, :], start=(v == 0), stop=False)
                        for k in range(KC):
                            nc.tensor.matmul(p0[:], w_whh0[:, k, ts(m, 128)],
                                             h0f[:, k, :], start=False,
                                             stop=(k == KC - 1))
                        nc.scalar.activation(h0s[:, m, :], p0[:],
                                             mybir.ActivationFunctionType.Tanh,
                                             bias=b0_sb[:, m:m + 1])
                        if tau == T - 1 and not hf_done[0]:
                            hf0 = sp.tile([128, SC, B], F32, tag="hf0", name="hf0") \
                                if m == 0 else hf0
                            nc.scalar.activation(hf0[:, m, :], p0[:],
                                                 mybir.ActivationFunctionType.Tanh,
                                                 bias=b0_sb[:, m:m + 1])
                    if tau == T - 1:
                        nc.sync.dma_start(
                            hfin_out[0].rearrange("(ci p) b -> p ci b", p=128), hf0[:])
                        hf_done[0] = True

                # -- L1: h1_{tau-1} --
                if 1 <= tau <= T:
                    h1s = sp.tile([128, SC, B], F16, tag="h1s", name="h1s")
                    for m in range(SC):
                        p1 = pp.tile([128, B], F32, tag="ps_l1", name="ps_l1")
                        for k in range(KC):
                            nc.tensor.matmul(p1[:], w_wih1[:, k, ts(m, 128)],
                                             h0f[:, k, :], start=(k == 0), stop=False)
                        for k in range(KC):
                            nc.tensor.matmul(p1[:], w_whh1[:, k, ts(m, 128)],
                                             h1f[:, k, :], start=False,
                                             stop=(k == KC - 1))
                        nc.scalar.activation(h1s[:, m, :], p1[:],
                                             mybir.ActivationFunctionType.Tanh,
                                             bias=b1_sb[:, m:m + 1])
                        if tau == T and not hf_done[1]:
                            hf1 = sp.tile([128, SC, B], F32, tag="hf1", name="hf1") \
                                if m == 0 else hf1
                            nc.scalar.activation(hf1[:, m, :], p1[:],
                                                 mybir.ActivationFunctionType.Tanh,
                                                 bias=b1_sb[:, m:m + 1])
                    if tau == T:
                        nc.sync.dma_start(
                            hfin_out[1].rearrange("(ci p) b -> p ci b", p=128), hf1[:])
                        hf_done[1] = True

                # -- fused AllGather: [h0_tau ; h1_{tau-1}] --
                if tau <= T:
                    agin = dp.tile([2 * S, B // 2], F32, tag="agin", name="agin")
                    nc.sync.dma_start(
                        agin[0:S].rearrange("(ci p) n -> p ci n", p=128),
                        h0s.bitcast(F32))
                    nc.sync.dma_start(
                        agin[S:2 * S].rearrange("(ci p) n -> p ci n", p=128),
                        h1s.bitcast(F32))
                    agout = dp.tile([NC * 2 * S, B // 2], F32, addr_space="Shared",
                                    tag="agout", name="agout")
                    nc.gpsimd.collective_compute(
                        "AllGather", mybir.AluOpType.bypass,
                        ins=[agin.opt()], outs=[agout.opt()], replica_groups=rg)
                    agout_prev = agout

                # -- prefetch next one-hot (DVE, overlaps AG) --
                if tau + 1 <= T - 1:
                    oh_cur = build_oh(tau + 1)

                # -- in-loop logits for step tau-2 (PE during AG flight) --
                if 2 <= tau <= T + 1:
                    plg = pp.tile([VS, B], F32, tag="ps_lg", name="ps_lg")
                    for k in range(KC):
                        nc.tensor.matmul(plg[:], w_fcw[:, k, :], h1f[:, k, :],
                                         start=(k == 0), stop=(k == KC - 1))
                    lg = sp.tile([VS, B], F32, tag="lg", name="lg")
                    nc.vector.tensor_tensor(lg[:], plg[:],
                                            fb_sb[:, 0:1].to_broadcast((VS, B)),
                                            mybir.AluOpType.add)
                    nc.sync.dma_start(logits_out[tau - 2], lg[:])

    if legalize:
        _legalize_waits(nc, max_waits=1)
    return nc


# ------------------------------------------------------------ host side ----

_RUNNER_CACHE = {}


def _get_runner(T=T_FULL):
    if T not in _RUNNER_CACHE:
        nc = build_nc(T)
        _RUNNER_CACHE[T] = _make_runner(nc, NC)
    return _RUNNER_CACHE[T]


def make_in_maps(tokens, h0, embedding, W_ih, W_hh, b_ih, b_hh, fc_w, fc_b, T=T_FULL):
    tokens = np.asarray(tokens)
    h0 = np.asarray(h0, np.float32)
    embedding = np.asarray(embedding, np.float32)
    W_ih = np.asarray(W_ih, np.float32)
    W_hh = np.asarray(W_hh, np.float32)
    b_ih = np.asarray(b_ih, np.float32)
    b_hh = np.asarray(b_hh, np.float32)
    fc_w = np.asarray(fc_w, np.float32)
    fc_b = np.asarray(fc_b, np.float32)

    embt = np.ascontiguousarray(embedding.T).astype(NPF16)
    hin0 = np.ascontiguousarray(h0[0].T).astype(NPF16)
    hin1 = np.ascontiguousarray(h0[1].T).astype(NPF16)
    tokf = np.broadcast_to(
        tokens.T[:T].astype(np.float16)[:, None, :], (T, 128, B)).copy()
    iotat = (np.arange(128, dtype=np.float16)[:, None]
             + np.float16(128.0) * np.arange(VC, dtype=np.float16)[None, :])
    b0 = b_ih[0] + b_hh[0]
    b1 = b_ih[1] + b_hh[1]

    in_maps = []
    for c in range(NC):
        sl = slice(c * S, (c + 1) * S)
        vsl = slice(c * VS, (c + 1) * VS)
        in_maps.append({
            "whh0t": np.ascontiguousarray(W_hh[0][sl, :].T).astype(NPF16),
            "wih1t": np.ascontiguousarray(W_ih[1][sl, :].T).astype(NPF16),
            "whh1t": np.ascontiguousarray(W_hh[1][sl, :].T).astype(NPF16),
            "wih0t": np.ascontiguousarray(W_ih[0][sl, :].T).astype(NPF16),
            "embt": embt,
            "fcwt": np.ascontiguousarray(fc_w[vsl, :].T).astype(NPF16),
            "b0c": np.ascontiguousarray(b0[sl].reshape(SC, 128).T).astype(np.float32),
            "b1c": np.ascontiguousarray(b1[sl].reshape(SC, 128).T).astype(np.float32),
            "fbc": np.ascontiguousarray(fc_b[vsl].reshape(VS, 1)).astype(np.float32),
            "hin0": hin0,
            "hin1": hin1,
            "hin1s": np.ascontiguousarray(
                hin1[sl].reshape(SC, 128, B).transpose(1, 0, 2)).astype(NPF16),
            "tokf": tokf,
            "iotat": iotat,
        })
    return in_maps


def assemble_outputs(results, T=T_FULL):
    logits_t_vb = np.concatenate(
        [results[c]["logits_out"] for c in range(NC)], axis=1)  # [T, V, B]
    logits = np.ascontiguousarray(logits_t_vb.transpose(0, 2, 1))  # [T, B, V]
    hfin_hb = np.concatenate(
        [results[c]["hfin_out"] for c in range(NC)], axis=1)  # [2, H, B]
    h_final = np.ascontiguousarray(hfin_hb.transpose(0, 2, 1))  # [2, B, H]
    return logits, h_final


def kernel(tokens, h0, embedding, W_ih, W_hh, b_ih, b_hh, fc_w, fc_b):
    fn = _get_runner(T_FULL)
    in_maps = make_in_maps(tokens, h0, embedding, W_ih, W_hh,
                           b_ih, b_hh, fc_w, fc_b, T=T_FULL)
    results = fn(in_maps)
    return assemble_outputs(results, T=T_FULL)
